# revision 30
# baseline (speedup 1.0000x reference)
"""BiAffine layer kernel for 8 Trainium2 NeuronCores.

Reference computation (per batch b):
  s = relu(x @ sW.T + sb)                  [L, E]
  t = relu(x @ tW.T + tb)                  [L, E]
  key = (s @ blW.T).reshape(L, E, N)
  out1[i, n, l] = sum_e key[i, e, n] * t[l, e]
  su = s @ Wu.T + f2b ; tv = t @ Wv.T      (Wu, Wv = f2W[:, :E], f2W[:, E:])
  h[i, j, :] = relu(su[i] + tv[j])
  out2[i, n, j] = sum_e h[i, j, e] * f3W[n, e] + f3b[n]
  out = out1 + out2                        [L, N, L]

Sharding: 8 cores = 2 batches x 4 blocks of 128 source positions (i).

Octet layout: one PSUM bank [128, 512] holds EIGHT i's: 4 col-groups at
32-aligned offsets, 2 i's packed per group (rows 32k + 12s + n, 8 pad
rows per group).  out1: M=32 matmuls from a zero-padded fp16 key tensor
(also initializes the bank); out2: M=24 matmuls with zero-block-padded
f3W stationaries, 4-way PE column-group concurrency.

h production per octet position p = i%8:
  p=0..5 -> DVE: ONE batched fp16 tensor_tensor per octet computing
    h' = max(tv, -su') for all six i's ([128, 6x512x2] interleaved
    layout, stride-0 broadcast APs; 2x perf mode, ~540ns/i).  The
    dropped +su' makes the matmul short by sum_e f3W[n,e]su'[e,i]; that
    rank-1 correction C is precomputed on the PE and folded (with f3b)
    into the final copy's per-partition bias.
  p=6,7 -> ACT: true h = relu(tv + su') via activation bias, reading tv
    from a persistent PSUM bank (ScalarE's fast src).
Final: one biased copy [128,512] per octet (ACT on even octets, DVE
tensor_scalar-add on odd), then ONE output DMA per octet into a padded
[OCTS, 128, L] fp16 dram layout (host strips the pad rows), issued on
the sync HWDGE ring.  GpSimd does no DMA (SWDGE issue is 2-14us for
large transfers); all input loads ride the sync/scalar HWDGE rings,
s-path tensors first so prep matmuls start during the load tail.
"""

import sys

sys.path.insert(0, "/opt/trn_rl_repo")

import numpy as np

B, L, H, E, N = 2, 512, 768, 256, 12
EC = E // 128  # 2 e-chunks
HC = H // 128  # 6 h-chunks
IB = L // 4  # 128 i's per core
NCORES = 8
OCTS = IB // 8  # 16

# misc fp32 column layout: [sb(2) tb(2) f2b(2) f3b128(1) kxn01(16)
#                           mask8(8) f3WT(24)]
MISC_W = 2 + 2 + 2 + 1 + OCTS + 8 + 2 * N

_cache = {}


def build_nc():
    import concourse.bass as bass
    import concourse.tile as tile
    from concourse import bacc, mybir
    from contextlib import ExitStack

    fp32 = mybir.dt.float32
    fp16 = mybir.dt.float16
    AF = mybir.ActivationFunctionType
    ALU = mybir.AluOpType

    nc = bacc.Bacc("TRN2")

    # ---- I/O (multi-chunk tensors prepacked chunk-major on host) ----
    xTm = nc.dram_tensor("xTm", [128, HC * L], fp16, kind="ExternalInput")
    xTim = nc.dram_tensor("xTim", [128, HC * IB], fp16, kind="ExternalInput")
    sWTm = nc.dram_tensor("sWTm", [128, HC * E], fp16, kind="ExternalInput")
    tWTm = nc.dram_tensor("tWTm", [128, HC * E], fp16, kind="ExternalInput")
    WuTm = nc.dram_tensor("WuTm", [128, EC * E], fp16, kind="ExternalInput")
    WvTm = nc.dram_tensor("WvTm", [128, EC * E], fp16, kind="ExternalInput")
    blWTm = nc.dram_tensor("blWTm", [128, EC * E * N], fp16, kind="ExternalInput")
    f3padm = nc.dram_tensor("f3padm", [128, EC * 48], fp16, kind="ExternalInput")
    misc = nc.dram_tensor("misc", [128, MISC_W], fp32, kind="ExternalInput")
    out = nc.dram_tensor("out", [OCTS, 128, L], fp16, kind="ExternalOutput")

    with tile.TileContext(nc) as tc, ExitStack() as ctx:
        consts = ctx.enter_context(tc.tile_pool(name="consts", bufs=1))
        acts = ctx.enter_context(tc.tile_pool(name="acts", bufs=1))
        tvps = ctx.enter_context(tc.tile_pool(name="tvps", bufs=1, space="PSUM"))

        def load(src, shape, name, dt=fp16, eng=None):
            t = consts.tile(shape, dt, name=name)
            (eng or nc.sync).dma_start(out=t[:], in_=src)
            return t

        # HWDGE rings only; s-path first so prep compute starts early.
        # blW (the biggest tensor) is split across both rings mid-order.
        # t-chain first: tv gates the DVE h stream, the kernel's cadence.
        # xT leads the sync ring (before even misc) so its transfer
        # starts the moment the ring opens.
        # xT arrives as 4 l-block quarters (lb-major, hc within) so the
        # t-chain pipelines under the load tail
        xT_m = consts.tile([128, HC * L], fp16, name="xT_m")
        for lb in range(4):
            nc.sync.dma_start(out=xT_m[:, 768 * lb : 768 * (lb + 1)],
                              in_=xTm[:, 768 * lb : 768 * (lb + 1)])
        misc_sb = load(misc[:], [128, MISC_W], "misc_sb", dt=fp32, eng=nc.sync)
        tWT_m = load(tWTm[:], [128, HC * E], "tWT_m", eng=nc.scalar)
        xTi_m = load(xTim[:], [128, HC * IB], "xTi_m", eng=nc.scalar)
        sWT_m = load(sWTm[:], [128, HC * E], "sWT_m", eng=nc.scalar)
        WvT_m = load(WvTm[:], [128, EC * E], "WvT_m", eng=nc.sync)
        WuT_m = load(WuTm[:], [128, EC * E], "WuT_m", eng=nc.scalar)
        blWT_m = consts.tile([128, EC * E * N], fp16, name="blWT_m")
        half = EC * E * N // 2
        nc.sync.dma_start(out=blWT_m[:, :half], in_=blWTm[:, :half])
        nc.scalar.dma_start(out=blWT_m[:, half:], in_=blWTm[:, half:])
        f3pad_m = load(f3padm[:], [128, EC * 48], "f3pad_m", eng=nc.sync)

        # xT blocks indexed 6*lb + hc, each [128, 128]
        xT_sb = [xT_m[:, 128 * c : 128 * (c + 1)] for c in range(4 * HC)]
        xTi_sb = [xTi_m[:, IB * c : IB * (c + 1)] for c in range(HC)]
        sWT_sb = [sWT_m[:, E * c : E * (c + 1)] for c in range(HC)]
        tWT_sb = [tWT_m[:, E * c : E * (c + 1)] for c in range(HC)]
        WuT_sb = [WuT_m[:, E * c : E * (c + 1)] for c in range(EC)]
        WvT_sb = [WvT_m[:, E * c : E * (c + 1)] for c in range(EC)]
        blWT_sb = [blWT_m[:, E * N * c : E * N * (c + 1)] for c in range(EC)]
        f3pad_sb = [f3pad_m[:, 48 * c : 48 * (c + 1)] for c in range(EC)]
        o_ = 0
        sb_sb = misc_sb[:, o_ : o_ + 2]; o_ += 2
        tb_sb = misc_sb[:, o_ : o_ + 2]; o_ += 2
        f2b_sb = misc_sb[:, o_ : o_ + 2]; o_ += 2
        f3b_sb = misc_sb[:, o_ : o_ + 1]; o_ += 1
        kxn01_sb = misc_sb[:, o_ : o_ + OCTS]; o_ += OCTS
        mask8_sb = misc_sb[:, o_ : o_ + 8]; o_ += 8
        f3WT_sb = [misc_sb[:, o_ + N * c : o_ + N * (c + 1)] for c in range(EC)]

        # ---- persistent activations ----
        tT_sb, sT_sb, suT_sb, keyE_sb, tvPS = [], [], [], [], []
        for ec in range(EC):
            tT_sb.append(acts.tile([128, L], fp16, name=f"tT{ec}"))
            sT_sb.append(acts.tile([128, IB], fp16, name=f"sT{ec}"))
            suT_sb.append(acts.tile([128, IB], fp32, name=f"suT{ec}"))
            # key, n-major contiguous: col 128*n + i; out1 reads it via a
            # gathered [12 n, 2 i] stationary AP (no scatter, no pads)
            keyE_sb.append(acts.tile([128, N * IB], fp16, name=f"keyE_{ec}"))
            tvPS.append(tvps.tile([128, L], fp32, name=f"tvPS{ec}"))
        tvT2i = acts.tile([128, 2 * L], fp16, name="tvT2i")  # cols 2*j+ec
        negsu2 = acts.tile([128, 2 * IB], fp16, name="negsu2")  # col 2i+ec
        C8sb = acts.tile([128, OCTS], fp32, name="C8sb")
        CT_sb = acts.tile([128, N], fp32, name="CT_sb")
        kxmC = acts.tile([128, 128], fp32, name="kxmC")
        nc.gpsimd.memset(kxmC[:], 0.0)

        # ---- prep ----
        with tc.tile_pool(name="prep_psum", bufs=3, space="PSUM") as pp:
            # PE warmup: junk matmuls while input DMAs land, so HAM
            # unthrottles (1.2 -> 2.4 GHz) before real prep work arrives
            scratch = acts.tile([128, L], fp16, name="scratch")
            nc.gpsimd.memset(scratch[:], 0.0)
            ps_w = pp.tile([128, L], fp32, name="ps_w", tag="ps")
            for w in range(6):
                nc.tensor.matmul(
                    ps_w[:], lhsT=scratch[:, :128], rhs=scratch[:],
                    start=(w == 0), stop=(w == 5))

            # s-chain first: its inputs (xTi, sW on the scalar ring) land
            # before xT's 786KB finishes streaming
            for ec in range(EC):
                ps_s = pp.tile([128, L], fp32, name="ps_s", tag="ps")
                for hc in range(HC):
                    nc.tensor.matmul(
                        ps_s[:, :IB],
                        lhsT=sWT_sb[hc][:, 128 * ec : 128 * (ec + 1)],
                        rhs=xTi_sb[hc],
                        start=(hc == 0),
                        stop=(hc == HC - 1),
                    )
                nc.scalar.activation(sT_sb[ec][:], ps_s[:, :IB], AF.Relu,
                                     bias=sb_sb[:, ec : ec + 1])
            for ec in range(EC):
                # suT = s @ Wu.T + f2b (fp32; per-partition biases/scalars)
                ps_su = pp.tile([128, L], fp32, name="ps_su", tag="ps")
                for epc in range(EC):
                    nc.tensor.matmul(
                        ps_su[:, :IB],
                        lhsT=WuT_sb[epc][:, 128 * ec : 128 * (ec + 1)],
                        rhs=sT_sb[epc][:],
                        start=(epc == 0),
                        stop=(epc == EC - 1),
                    )
                nc.scalar.activation(suT_sb[ec][:], ps_su[:, :IB], AF.Identity,
                                     bias=f2b_sb[:, ec : ec + 1])
                ns2 = negsu2.rearrange("p (i two) -> p i two", two=2)
                nc.vector.tensor_scalar_mul(ns2[:, :, ec], suT_sb[ec][:], -1.0)

            # t-chain, pipelined per 128-col l-block as xT quarters land:
            # t matmuls -> tv matmuls -> interleaved cast, all per block
            tv_i = tvT2i.rearrange("p (f c) -> p f c", c=2)
            ps_t = [pp.tile([128, L], fp32, name=f"ps_t{ec}", tag="ps")
                    for ec in range(EC)]
            for lb in range(4):
                sl = slice(128 * lb, 128 * (lb + 1))
                for ec in range(EC):
                    for hc in range(HC):
                        nc.tensor.matmul(
                            ps_t[ec][:, sl],
                            lhsT=tWT_sb[hc][:, 128 * ec : 128 * (ec + 1)],
                            rhs=xT_sb[6 * lb + hc],
                            start=(hc == 0),
                            stop=(hc == HC - 1),
                        )
                    nc.scalar.activation(tT_sb[ec][:, sl], ps_t[ec][:, sl],
                                         AF.Relu, bias=tb_sb[:, ec : ec + 1])
                for ec in range(EC):
                    for epc in range(EC):
                        nc.tensor.matmul(
                            tvPS[ec][:, sl],
                            lhsT=WvT_sb[epc][:, 128 * ec : 128 * (ec + 1)],
                            rhs=tT_sb[epc][:, sl],
                            start=(epc == 0),
                            stop=(epc == EC - 1),
                        )
                    nc.vector.tensor_copy(out=tv_i[:, sl, ec],
                                          in_=tvPS[ec][:, sl])

            # correction CT[i, n] = sum_e f3WT[e,n] * suT[e,i]  (fp32)
            ps_ct = pp.tile([128, L], fp32, name="ps_ct", tag="ps")
            for ec in range(EC):
                nc.tensor.matmul(
                    ps_ct[:, :N],
                    lhsT=suT_sb[ec][:],
                    rhs=f3WT_sb[ec],
                    start=(ec == 0),
                    stop=(ec == EC - 1),
                )
            nc.vector.tensor_copy(out=CT_sb[:], in_=ps_ct[:, :N])
            # kxmC[:, 32k+12s+n] = CT[:, n] * (i%8 == 2k+s), p<6 only
            for k in range(4):
                for s in range(2):
                    p = 2 * k + s
                    if p >= 6:
                        continue
                    nc.vector.tensor_tensor(
                        out=kxmC[:, 32 * k + 12 * s : 32 * k + 12 * s + N],
                        in0=CT_sb[:],
                        in1=mask8_sb[:, p : p + 1].broadcast_to([128, N]),
                        op=ALU.mult,
                    )
            ps_c8 = pp.tile([128, L], fp32, name="ps_c8", tag="ps")
            nc.tensor.matmul(ps_c8[:, :OCTS], lhsT=kxmC[:], rhs=kxn01_sb,
                             start=True, stop=True)
            nc.vector.tensor_tensor(
                out=C8sb[:], in0=ps_c8[:, :OCTS],
                in1=f3b_sb.broadcast_to([128, OCTS]), op=ALU.add)

        # ---- main loop over octets ----
        # Key production (48 matmuls + 24 strided copies) is interleaved
        # into the first KLAG octets so the DVE h stream never waits on
        # it: those octets accumulate out2 FIRST (start=True, no key
        # dependency) and their out1 joins KLAG octets later
        # (start=False, stop=True).  Octets > KLAG run out1-first
        # (v0-style, start=True incl pad rows) and flush immediately.
        hp = ctx.enter_context(tc.tile_pool(name="hp", bufs=12))
        outp = ctx.enter_context(tc.tile_pool(name="outp", bufs=4))
        mp = ctx.enter_context(tc.tile_pool(name="main_psum", bufs=5, space="PSUM"))
        kp = ctx.enter_context(tc.tile_pool(name="key_psum", bufs=1, space="PSUM"))

        tv_i = tvT2i.rearrange("p (f c) -> p f c", c=2)
        ns2 = negsu2.rearrange("p (i two) -> p i two", two=2)
        blWT3 = [blWT_sb[c].rearrange("p (e n) -> p e n", n=N) for c in range(EC)]
        # key layout: keyF[ec][e, 64*(12s+n) + d]  (i = 2d+s) -> out1's
        # stationary for duo d is a clean 2D AP [24 cols, stride 64]
        keyv = [keyE_sb[c].rearrange("p (c d) -> p c d", d=64)
                for c in range(EC)]
        keyw = [keyE_sb[c].rearrange("p (s n d) -> p s n d", s=2, d=64)
                for c in range(EC)]
        # s-major rhs: key matmul emits cols 64s+d so the copy is
        # contiguous-inner on both sides
        sT_sm = [sT_sb[c].rearrange("p (d s) -> p s d", s=2) for c in range(EC)]
        # 6 key groups of (ec, 4 consecutive n): one PSUM bank + 1 copy each
        KEY_GROUPS = [(ec, 4 * q) for ec in range(EC) for q in range(3)]
        KLAG = 4
        pending = []  # (psum_tile, octet) awaiting lagged out1 + flush

        def key_batch(groups):
            for gi, (ec, n0) in enumerate(groups):
                ps_k = kp.tile([128, L], fp32, name="ps_k", tag="psk")
                for q in range(4):
                    for epc in range(EC):
                        nc.tensor.matmul(
                            ps_k[:, IB * q : IB * (q + 1)],
                            lhsT=blWT3[epc][:, 128 * ec : 128 * (ec + 1),
                                            n0 + q],
                            rhs=sT_sm[epc],
                            start=(epc == 0),
                            stop=(epc == EC - 1),
                        )
                src = ps_k[:].rearrange("p (q s d) -> p s q d", q=4, s=2)
                dst = keyw[ec][:, :, n0 : n0 + 4, :]
                if gi % 2 == 0:
                    nc.vector.tensor_copy(out=dst, in_=src)
                else:
                    nc.scalar.copy(dst, src)

        def emit_out1(ps_o, o, lagged):
            for ec in range(EC):
                for k in range(4):
                    d = 4 * o + k
                    nc.tensor.matmul(
                        ps_o[32 * k : 32 * k + 24, :],
                        lhsT=keyv[ec][:, :, d : d + 1],
                        rhs=tT_sb[ec][:],
                        start=(not lagged and ec == 0),
                        stop=(lagged and ec == EC - 1),
                        tile_position=(0, 32 * k),
                        skip_group_check=True,
                    )

        def flush(ps_o, o):
            ob = outp.tile([128, L], fp16, name="ob")
            if o == OCTS - 1:
                # DVE is idle after its last h6; shaves the tail
                nc.vector.tensor_scalar(
                    out=ob[:], in0=ps_o[:], scalar1=C8sb[:, o : o + 1],
                    scalar2=None, op0=ALU.add)
            else:
                nc.scalar.activation(ob[:], ps_o[:], AF.Identity,
                                     bias=C8sb[:, o : o + 1])
            nc.sync.dma_start(out=out[o], in_=ob[:])

        for o in range(OCTS):
            ps = mp.tile([128, L], fp32, name="ps")
            early = o <= KLAG
            # h: p=0..5 one batched DVE op (h'); p=6,7 ACT (true h, PSUM src)
            i0 = 8 * o
            h6 = hp.tile([128, 6 * 2 * L], fp16, name="h6", tag="h6", bufs=6)
            h6v = h6.rearrange("p (i f c) -> p i f c", f=L, c=2)
            nc.vector.tensor_tensor(
                out=h6v,
                in0=tv_i.unsqueeze(1).broadcast_to([128, 6, L, 2]),
                in1=ns2[:, i0 : i0 + 6, :].unsqueeze(2).broadcast_to(
                    [128, 6, L, 2]),
                op=ALU.max)
            hs = {}
            for p in range(6):
                for ec in range(EC):
                    hs[(p, ec)] = h6v[:, p, :, ec]
            for p in (6, 7):
                i = i0 + p
                for ec in range(EC):
                    ha = hp.tile([128, L], fp16, name="ha", tag="ha", bufs=12)
                    nc.scalar.activation(ha[:], tvPS[ec][:], AF.Relu,
                                         bias=suT_sb[ec][:, i : i + 1])
                    hs[(p, ec)] = ha[:]
            if o < 3:
                key_batch(KEY_GROUPS[2 * o : 2 * o + 2])
            if not early:
                emit_out1(ps, o, lagged=False)
            # out2: M=24 zero-block-padded f3 stationaries; emission order
            # rotates col-groups for PE tile concurrency
            for ec in range(EC):
                for p in (0, 2, 4, 6, 1, 3, 5, 7):
                    k, s = divmod(p, 2)
                    nc.tensor.matmul(
                        ps[32 * k : 32 * k + 24, :],
                        lhsT=f3pad_sb[ec][:, 24 * s : 24 * s + 24],
                        rhs=hs[(p, ec)],
                        start=(early and ec == 0 and s == 0),
                        stop=(not early and ec == EC - 1 and s == 1),
                        tile_position=(0, 32 * k),
                        skip_group_check=True,
                    )
            if early:
                pending.append((ps, o))
                if o == KLAG:
                    ps0, o0 = pending.pop(0)
                    emit_out1(ps0, o0, lagged=True)
                    flush(ps0, o0)
            else:
                flush(ps, o)
                if pending:
                    ps0, o0 = pending.pop(0)
                    emit_out1(ps0, o0, lagged=True)
                    flush(ps0, o0)
        for ps0, o0 in pending:
            emit_out1(ps0, o0, lagged=True)
            flush(ps0, o0)

    nc.compile()
    return nc


def _get_nc():
    if "nc" not in _cache:
        _cache["nc"] = build_nc()
    return _cache["nc"]


def _chunk_major(a, nchunks):
    # [128*nchunks, W] -> [128, nchunks*W] with chunk-major free layout
    W = a.shape[1]
    return np.ascontiguousarray(
        a.reshape(nchunks, 128, W).transpose(1, 0, 2).reshape(128, nchunks * W))


def _make_in_maps(inputs):
    x = np.asarray(inputs["x"], np.float32)
    f32 = lambda a: np.asarray(a, np.float32)

    f2W = f32(inputs["f2W"])
    f3WT = f32(inputs["f3W"]).T  # [E, N]
    f3pad = np.zeros((E, 48), np.float32)
    for s in range(2):
        # slice s covers psum rows 32k..32k+24; i with s=i%2 lands at +12*s
        f3pad[:, 24 * s + 12 * s : 24 * s + 12 * s + N] = f3WT

    misc = np.zeros((128, MISC_W), np.float32)
    o_ = 0
    misc[:, o_ : o_ + 2] = f32(inputs["sb"]).reshape(EC, 128).T; o_ += 2
    misc[:, o_ : o_ + 2] = f32(inputs["tb"]).reshape(EC, 128).T; o_ += 2
    misc[:, o_ : o_ + 2] = f32(inputs["f2b"]).reshape(EC, 128).T; o_ += 2
    for k in range(4):
        for s in range(2):
            misc[32 * k + 12 * s : 32 * k + 12 * s + N, o_] = f32(inputs["f3b"])
    o_ += 1
    for i in range(128):
        if i % 8 < 6:
            misc[i, o_ + i // 8] = 1.0
    o_ += OCTS
    for i in range(128):
        misc[i, o_ + i % 8] = 1.0
    o_ += 8
    misc[:, o_:] = _chunk_major(f3WT, EC)

    shared = {
        "sWTm": _chunk_major(f32(inputs["sW"]).T, HC).astype(np.float16),
        "tWTm": _chunk_major(f32(inputs["tW"]).T, HC).astype(np.float16),
        "WuTm": _chunk_major(f2W[:, :E].T, EC).astype(np.float16),
        "WvTm": _chunk_major(f2W[:, E:].T, EC).astype(np.float16),
        "blWTm": _chunk_major(f32(inputs["blW"]).T, EC).astype(np.float16),
        "f3padm": _chunk_major(f3pad, EC).astype(np.float16),
        "misc": misc,
    }

    in_maps = []
    for c in range(NCORES):
        b, r = divmod(c, 4)
        m = dict(shared)
        # xT lb-major: [p, (lb, hc, 128)] so each quarter is one DMA
        xt = x[b].T.reshape(HC, 128, 4, 128)
        m["xTm"] = np.ascontiguousarray(
            xt.transpose(1, 2, 0, 3).reshape(128, HC * L)).astype(np.float16)
        m["xTim"] = _chunk_major(
            np.ascontiguousarray(x[b, IB * r : IB * (r + 1), :].T), HC
        ).astype(np.float16)
        in_maps.append(m)
    return in_maps


def _gather(results):
    full = np.empty((B, L, N, L), np.float32)
    for c in range(NCORES):
        b, r = divmod(c, 4)
        # out is [OCTS, 128, L]; row 32k+12s+n of octet o -> i = 8o+2k+s
        raw = results[c]["out"].astype(np.float32)
        v = raw.reshape(OCTS, 4, 32, L)[:, :, :24, :]
        full[b, IB * r : IB * (r + 1)] = v.reshape(IB, N, L)
    return full


def kernel(x, sW, sb, tW, tb, f2W, f2b, f3W, f3b, blW):
    from concourse.bass_utils import run_bass_kernel_spmd

    in_maps = _make_in_maps(dict(
        x=x, sW=sW, sb=sb, tW=tW, tb=tb, f2W=f2W, f2b=f2b,
        f3W=f3W, f3b=f3b, blW=blW,
    ))
    nc = _get_nc()
    res = run_bass_kernel_spmd(nc, in_maps, core_ids=list(range(NCORES)))
    return _gather(res.results)


# revision 34
# speedup vs baseline: 1.0115x; 1.0115x over previous
"""BiAffine layer kernel for 8 Trainium2 NeuronCores.

Reference computation (per batch b):
  s = relu(x @ sW.T + sb)                  [L, E]
  t = relu(x @ tW.T + tb)                  [L, E]
  key = (s @ blW.T).reshape(L, E, N)
  out1[i, n, l] = sum_e key[i, e, n] * t[l, e]
  su = s @ Wu.T + f2b ; tv = t @ Wv.T      (Wu, Wv = f2W[:, :E], f2W[:, E:])
  h[i, j, :] = relu(su[i] + tv[j])
  out2[i, n, j] = sum_e h[i, j, e] * f3W[n, e] + f3b[n]
  out = out1 + out2                        [L, N, L]

Sharding: 8 cores = 2 batches x 4 blocks of 128 source positions (i).

Octet layout: one PSUM bank [128, 512] holds EIGHT i's: 4 col-groups at
32-aligned offsets, 2 i's packed per group (rows 32k + 12s + n, 8 pad
rows per group).  out1: M=32 matmuls from a zero-padded fp16 key tensor
(also initializes the bank); out2: M=24 matmuls with zero-block-padded
f3W stationaries, 4-way PE column-group concurrency.

h production per octet position p = i%8:
  p=0..5 -> DVE: ONE batched fp16 tensor_tensor per octet computing
    h' = max(tv, -su') for all six i's ([128, 6x512x2] interleaved
    layout, stride-0 broadcast APs; 2x perf mode, ~540ns/i).  The
    dropped +su' makes the matmul short by sum_e f3W[n,e]su'[e,i]; that
    rank-1 correction C is precomputed on the PE and folded (with f3b)
    into the final copy's per-partition bias.
  p=6,7 -> ACT: true h = relu(tv + su') via activation bias, reading tv
    from a persistent PSUM bank (ScalarE's fast src).
Final: one biased copy [128,512] per octet (ACT on even octets, DVE
tensor_scalar-add on odd), then ONE output DMA per octet into a padded
[OCTS, 128, L] fp16 dram layout (host strips the pad rows), issued on
the sync HWDGE ring.  GpSimd does no DMA (SWDGE issue is 2-14us for
large transfers); all input loads ride the sync/scalar HWDGE rings,
s-path tensors first so prep matmuls start during the load tail.
"""

import sys

sys.path.insert(0, "/opt/trn_rl_repo")

import numpy as np

B, L, H, E, N = 2, 512, 768, 256, 12
EC = E // 128  # 2 e-chunks
HC = H // 128  # 6 h-chunks
IB = L // 4  # 128 i's per core
NCORES = 8
OCTS = IB // 8  # 16

# misc fp32 column layout: [sb(2) tb(2) f2b(2) f3b128(1) kxn01(16)
#                           mask8(8) f3WT(24)]
MISC_W = 2 + 2 + 2 + 1 + OCTS + 8 + 2 * N

_cache = {}


def build_nc():
    import concourse.bass as bass
    import concourse.tile as tile
    from concourse import bacc, mybir
    from contextlib import ExitStack

    fp32 = mybir.dt.float32
    fp16 = mybir.dt.float16
    AF = mybir.ActivationFunctionType
    ALU = mybir.AluOpType

    nc = bacc.Bacc("TRN2")

    # ---- I/O (multi-chunk tensors prepacked chunk-major on host) ----
    xTm = nc.dram_tensor("xTm", [128, HC * L], fp16, kind="ExternalInput")
    xTim = nc.dram_tensor("xTim", [128, HC * IB], fp16, kind="ExternalInput")
    sWTm = nc.dram_tensor("sWTm", [128, HC * E], fp16, kind="ExternalInput")
    tWTm = nc.dram_tensor("tWTm", [128, HC * E], fp16, kind="ExternalInput")
    WuTm = nc.dram_tensor("WuTm", [128, EC * E], fp16, kind="ExternalInput")
    WvTm = nc.dram_tensor("WvTm", [128, EC * E], fp16, kind="ExternalInput")
    blWTm = nc.dram_tensor("blWTm", [128, EC * E * N], fp16, kind="ExternalInput")
    f3padm = nc.dram_tensor("f3padm", [128, EC * 48], fp16, kind="ExternalInput")
    misc = nc.dram_tensor("misc", [128, MISC_W], fp32, kind="ExternalInput")
    out = nc.dram_tensor("out", [OCTS, 128, L], fp16, kind="ExternalOutput")

    with tile.TileContext(nc) as tc, ExitStack() as ctx:
        consts = ctx.enter_context(tc.tile_pool(name="consts", bufs=1))
        acts = ctx.enter_context(tc.tile_pool(name="acts", bufs=1))
        tvps = ctx.enter_context(tc.tile_pool(name="tvps", bufs=1, space="PSUM"))

        def load(src, shape, name, dt=fp16, eng=None):
            t = consts.tile(shape, dt, name=name)
            (eng or nc.sync).dma_start(out=t[:], in_=src)
            return t

        # HWDGE rings only; s-path first so prep compute starts early.
        # blW (the biggest tensor) is split across both rings mid-order.
        # t-chain first: tv gates the DVE h stream, the kernel's cadence.
        # xT leads the sync ring (before even misc) so its transfer
        # starts the moment the ring opens.
        xT_m = load(xTm[:], [128, HC * L], "xT_m", eng=nc.sync)
        misc_sb = load(misc[:], [128, MISC_W], "misc_sb", dt=fp32, eng=nc.sync)
        tWT_m = load(tWTm[:], [128, HC * E], "tWT_m", eng=nc.scalar)
        xTi_m = load(xTim[:], [128, HC * IB], "xTi_m", eng=nc.scalar)
        sWT_m = load(sWTm[:], [128, HC * E], "sWT_m", eng=nc.scalar)
        WvT_m = load(WvTm[:], [128, EC * E], "WvT_m", eng=nc.sync)
        WuT_m = load(WuTm[:], [128, EC * E], "WuT_m", eng=nc.scalar)
        blWT_m = consts.tile([128, EC * E * N], fp16, name="blWT_m")
        half = EC * E * N // 2
        nc.sync.dma_start(out=blWT_m[:, :half], in_=blWTm[:, :half])
        nc.scalar.dma_start(out=blWT_m[:, half:], in_=blWTm[:, half:])
        f3pad_m = load(f3padm[:], [128, EC * 48], "f3pad_m", eng=nc.sync)

        xT_sb = [xT_m[:, L * c : L * (c + 1)] for c in range(HC)]
        xTi_sb = [xTi_m[:, IB * c : IB * (c + 1)] for c in range(HC)]
        sWT_sb = [sWT_m[:, E * c : E * (c + 1)] for c in range(HC)]
        tWT_sb = [tWT_m[:, E * c : E * (c + 1)] for c in range(HC)]
        WuT_sb = [WuT_m[:, E * c : E * (c + 1)] for c in range(EC)]
        WvT_sb = [WvT_m[:, E * c : E * (c + 1)] for c in range(EC)]
        blWT_sb = [blWT_m[:, E * N * c : E * N * (c + 1)] for c in range(EC)]
        f3pad_sb = [f3pad_m[:, 48 * c : 48 * (c + 1)] for c in range(EC)]
        o_ = 0
        sb_sb = misc_sb[:, o_ : o_ + 2]; o_ += 2
        tb_sb = misc_sb[:, o_ : o_ + 2]; o_ += 2
        f2b_sb = misc_sb[:, o_ : o_ + 2]; o_ += 2
        f3b_sb = misc_sb[:, o_ : o_ + 1]; o_ += 1
        kxn01_sb = misc_sb[:, o_ : o_ + OCTS]; o_ += OCTS
        mask8_sb = misc_sb[:, o_ : o_ + 8]; o_ += 8
        f3WT_sb = [misc_sb[:, o_ + N * c : o_ + N * (c + 1)] for c in range(EC)]

        # ---- persistent activations ----
        tT_sb, sT_sb, suT_sb, keyE_sb, tvPS = [], [], [], [], []
        for ec in range(EC):
            tT_sb.append(acts.tile([128, L], fp16, name=f"tT{ec}"))
            sT_sb.append(acts.tile([128, IB], fp16, name=f"sT{ec}"))
            suT_sb.append(acts.tile([128, IB], fp32, name=f"suT{ec}"))
            # key, n-major contiguous: col 128*n + i; out1 reads it via a
            # gathered [12 n, 2 i] stationary AP (no scatter, no pads)
            keyE_sb.append(acts.tile([128, N * IB], fp16, name=f"keyE_{ec}"))
            tvPS.append(tvps.tile([128, L], fp32, name=f"tvPS{ec}"))
        tvT2i = acts.tile([128, 2 * L], fp16, name="tvT2i")  # cols 2*j+ec
        negsu2 = acts.tile([128, 2 * IB], fp16, name="negsu2")  # col 2i+ec
        C8sb = acts.tile([128, OCTS], fp32, name="C8sb")
        CT_sb = acts.tile([128, N], fp32, name="CT_sb")
        kxmC = acts.tile([128, 128], fp32, name="kxmC")
        nc.gpsimd.memset(kxmC[:], 0.0)

        # ---- prep ----
        with tc.tile_pool(name="prep_psum", bufs=3, space="PSUM") as pp:
            # PE warmup: junk matmuls while input DMAs land, so HAM
            # unthrottles (1.2 -> 2.4 GHz) before real prep work arrives
            scratch = acts.tile([128, L], fp16, name="scratch")
            nc.gpsimd.memset(scratch[:], 0.0)
            ps_w = pp.tile([128, L], fp32, name="ps_w", tag="ps")
            for w in range(6):
                nc.tensor.matmul(
                    ps_w[:], lhsT=scratch[:, :128], rhs=scratch[:],
                    start=(w == 0), stop=(w == 5))

            # s-chain first: its inputs (xTi, sW on the scalar ring) land
            # before xT's 786KB finishes streaming
            for ec in range(EC):
                ps_s = pp.tile([128, L], fp32, name="ps_s", tag="ps")
                for hc in range(HC):
                    nc.tensor.matmul(
                        ps_s[:, :IB],
                        lhsT=sWT_sb[hc][:, 128 * ec : 128 * (ec + 1)],
                        rhs=xTi_sb[hc],
                        start=(hc == 0),
                        stop=(hc == HC - 1),
                    )
                nc.scalar.activation(sT_sb[ec][:], ps_s[:, :IB], AF.Relu,
                                     bias=sb_sb[:, ec : ec + 1])
            for ec in range(EC):
                # suT = s @ Wu.T + f2b (fp32; per-partition biases/scalars)
                ps_su = pp.tile([128, L], fp32, name="ps_su", tag="ps")
                for epc in range(EC):
                    nc.tensor.matmul(
                        ps_su[:, :IB],
                        lhsT=WuT_sb[epc][:, 128 * ec : 128 * (ec + 1)],
                        rhs=sT_sb[epc][:],
                        start=(epc == 0),
                        stop=(epc == EC - 1),
                    )
                nc.scalar.activation(suT_sb[ec][:], ps_su[:, :IB], AF.Identity,
                                     bias=f2b_sb[:, ec : ec + 1])
                ns2 = negsu2.rearrange("p (i two) -> p i two", two=2)
                nc.vector.tensor_scalar_mul(ns2[:, :, ec], suT_sb[ec][:], -1.0)

            # second warmup burst: keeps HAM hot while waiting for xT so
            # the t-chain (the h-stream gate) runs at 2.4 GHz
            ps_w2 = pp.tile([128, L], fp32, name="ps_w2", tag="ps")
            for w in range(6):
                nc.tensor.matmul(
                    ps_w2[:], lhsT=scratch[:, :128], rhs=scratch[:],
                    start=(w == 0), stop=(w == 5))

            # t-chain (tT -> tv gates the main-loop DVE stream)
            tv_i = tvT2i.rearrange("p (f c) -> p f c", c=2)
            for ec in range(EC):
                ps_t = pp.tile([128, L], fp32, name="ps_t", tag="ps")
                for hc in range(HC):
                    nc.tensor.matmul(
                        ps_t[:],
                        lhsT=tWT_sb[hc][:, 128 * ec : 128 * (ec + 1)],
                        rhs=xT_sb[hc],
                        start=(hc == 0),
                        stop=(hc == HC - 1),
                    )
                nc.scalar.activation(tT_sb[ec][:], ps_t[:], AF.Relu,
                                     bias=tb_sb[:, ec : ec + 1])
            for ec in range(EC):
                for epc in range(EC):
                    nc.tensor.matmul(
                        tvPS[ec][:],
                        lhsT=WvT_sb[epc][:, 128 * ec : 128 * (ec + 1)],
                        rhs=tT_sb[epc][:],
                        start=(epc == 0),
                        stop=(epc == EC - 1),
                    )
                nc.vector.tensor_copy(out=tv_i[:, :, ec], in_=tvPS[ec][:])

            # correction CT[i, n] = sum_e f3WT[e,n] * suT[e,i]  (fp32)
            ps_ct = pp.tile([128, L], fp32, name="ps_ct", tag="ps")
            for ec in range(EC):
                nc.tensor.matmul(
                    ps_ct[:, :N],
                    lhsT=suT_sb[ec][:],
                    rhs=f3WT_sb[ec],
                    start=(ec == 0),
                    stop=(ec == EC - 1),
                )
            nc.vector.tensor_copy(out=CT_sb[:], in_=ps_ct[:, :N])
            # kxmC[:, 32k+12s+n] = CT[:, n] * (i%8 == 2k+s), p<6 only
            for k in range(4):
                for s in range(2):
                    p = 2 * k + s
                    if p >= 6:
                        continue
                    nc.vector.tensor_tensor(
                        out=kxmC[:, 32 * k + 12 * s : 32 * k + 12 * s + N],
                        in0=CT_sb[:],
                        in1=mask8_sb[:, p : p + 1].broadcast_to([128, N]),
                        op=ALU.mult,
                    )
            ps_c8 = pp.tile([128, L], fp32, name="ps_c8", tag="ps")
            nc.tensor.matmul(ps_c8[:, :OCTS], lhsT=kxmC[:], rhs=kxn01_sb,
                             start=True, stop=True)
            nc.vector.tensor_tensor(
                out=C8sb[:], in0=ps_c8[:, :OCTS],
                in1=f3b_sb.broadcast_to([128, OCTS]), op=ALU.add)

        # ---- main loop over octets ----
        # Key production (48 matmuls + 24 strided copies) is interleaved
        # into the first KLAG octets so the DVE h stream never waits on
        # it: those octets accumulate out2 FIRST (start=True, no key
        # dependency) and their out1 joins KLAG octets later
        # (start=False, stop=True).  Octets > KLAG run out1-first
        # (v0-style, start=True incl pad rows) and flush immediately.
        hp = ctx.enter_context(tc.tile_pool(name="hp", bufs=12))
        outp = ctx.enter_context(tc.tile_pool(name="outp", bufs=4))
        mp = ctx.enter_context(tc.tile_pool(name="main_psum", bufs=5, space="PSUM"))
        kp = ctx.enter_context(tc.tile_pool(name="key_psum", bufs=1, space="PSUM"))

        tv_i = tvT2i.rearrange("p (f c) -> p f c", c=2)
        ns2 = negsu2.rearrange("p (i two) -> p i two", two=2)
        blWT3 = [blWT_sb[c].rearrange("p (e n) -> p e n", n=N) for c in range(EC)]
        # key layout: keyF[ec][e, 64*(12s+n) + d]  (i = 2d+s) -> out1's
        # stationary for duo d is a clean 2D AP [24 cols, stride 64]
        keyv = [keyE_sb[c].rearrange("p (c d) -> p c d", d=64)
                for c in range(EC)]
        keyw = [keyE_sb[c].rearrange("p (s n d) -> p s n d", s=2, d=64)
                for c in range(EC)]
        # s-major rhs: key matmul emits cols 64s+d so the copy is
        # contiguous-inner on both sides
        sT_sm = [sT_sb[c].rearrange("p (d s) -> p s d", s=2) for c in range(EC)]
        # 6 key groups of (ec, 4 consecutive n): one PSUM bank + 1 copy each
        KEY_GROUPS = [(ec, 4 * q) for ec in range(EC) for q in range(3)]
        KLAG = 4
        pending = []  # (psum_tile, octet) awaiting lagged out1 + flush

        def key_batch(groups):
            for gi, (ec, n0) in enumerate(groups):
                ps_k = kp.tile([128, L], fp32, name="ps_k", tag="psk")
                for q in range(4):
                    for epc in range(EC):
                        nc.tensor.matmul(
                            ps_k[:, IB * q : IB * (q + 1)],
                            lhsT=blWT3[epc][:, 128 * ec : 128 * (ec + 1),
                                            n0 + q],
                            rhs=sT_sm[epc],
                            start=(epc == 0),
                            stop=(epc == EC - 1),
                        )
                src = ps_k[:].rearrange("p (q s d) -> p s q d", q=4, s=2)
                dst = keyw[ec][:, :, n0 : n0 + 4, :]
                if gi % 2 == 0:
                    nc.vector.tensor_copy(out=dst, in_=src)
                else:
                    nc.scalar.copy(dst, src)

        def emit_out1(ps_o, o, lagged):
            for ec in range(EC):
                for k in range(4):
                    d = 4 * o + k
                    nc.tensor.matmul(
                        ps_o[32 * k : 32 * k + 24, :],
                        lhsT=keyv[ec][:, :, d : d + 1],
                        rhs=tT_sb[ec][:],
                        start=(not lagged and ec == 0),
                        stop=(lagged and ec == EC - 1),
                        tile_position=(0, 32 * k),
                        skip_group_check=True,
                    )

        def flush(ps_o, o):
            ob = outp.tile([128, L], fp16, name="ob")
            if o == OCTS - 1:
                # DVE is idle after its last h6; shaves the tail
                nc.vector.tensor_scalar(
                    out=ob[:], in0=ps_o[:], scalar1=C8sb[:, o : o + 1],
                    scalar2=None, op0=ALU.add)
            else:
                nc.scalar.activation(ob[:], ps_o[:], AF.Identity,
                                     bias=C8sb[:, o : o + 1])
            nc.sync.dma_start(out=out[o], in_=ob[:])

        for o in range(OCTS):
            ps = mp.tile([128, L], fp32, name="ps")
            early = o <= KLAG
            # h: p=0..5 one batched DVE op (h'); p=6,7 ACT (true h, PSUM src)
            i0 = 8 * o
            h6 = hp.tile([128, 6 * 2 * L], fp16, name="h6", tag="h6", bufs=6)
            h6v = h6.rearrange("p (i f c) -> p i f c", f=L, c=2)
            nc.vector.tensor_tensor(
                out=h6v,
                in0=tv_i.unsqueeze(1).broadcast_to([128, 6, L, 2]),
                in1=ns2[:, i0 : i0 + 6, :].unsqueeze(2).broadcast_to(
                    [128, 6, L, 2]),
                op=ALU.max)
            hs = {}
            for p in range(6):
                for ec in range(EC):
                    hs[(p, ec)] = h6v[:, p, :, ec]
            for p in (6, 7):
                i = i0 + p
                for ec in range(EC):
                    ha = hp.tile([128, L], fp16, name="ha", tag="ha", bufs=12)
                    nc.scalar.activation(ha[:], tvPS[ec][:], AF.Relu,
                                         bias=suT_sb[ec][:, i : i + 1])
                    hs[(p, ec)] = ha[:]
            if o < 3:
                key_batch(KEY_GROUPS[2 * o : 2 * o + 2])
            if not early:
                emit_out1(ps, o, lagged=False)
            # out2: M=24 zero-block-padded f3 stationaries; emission order
            # rotates col-groups for PE tile concurrency
            for ec in range(EC):
                for p in (0, 2, 4, 6, 1, 3, 5, 7):
                    k, s = divmod(p, 2)
                    nc.tensor.matmul(
                        ps[32 * k : 32 * k + 24, :],
                        lhsT=f3pad_sb[ec][:, 24 * s : 24 * s + 24],
                        rhs=hs[(p, ec)],
                        start=(early and ec == 0 and s == 0),
                        stop=(not early and ec == EC - 1 and s == 1),
                        tile_position=(0, 32 * k),
                        skip_group_check=True,
                    )
            if early:
                pending.append((ps, o))
                if o == KLAG:
                    ps0, o0 = pending.pop(0)
                    emit_out1(ps0, o0, lagged=True)
                    flush(ps0, o0)
            else:
                flush(ps, o)
                if pending:
                    ps0, o0 = pending.pop(0)
                    emit_out1(ps0, o0, lagged=True)
                    flush(ps0, o0)
        for ps0, o0 in pending:
            emit_out1(ps0, o0, lagged=True)
            flush(ps0, o0)

    nc.compile()
    return nc


def _get_nc():
    if "nc" not in _cache:
        _cache["nc"] = build_nc()
    return _cache["nc"]


def _chunk_major(a, nchunks):
    # [128*nchunks, W] -> [128, nchunks*W] with chunk-major free layout
    W = a.shape[1]
    return np.ascontiguousarray(
        a.reshape(nchunks, 128, W).transpose(1, 0, 2).reshape(128, nchunks * W))


def _make_in_maps(inputs):
    x = np.asarray(inputs["x"], np.float32)
    f32 = lambda a: np.asarray(a, np.float32)

    f2W = f32(inputs["f2W"])
    f3WT = f32(inputs["f3W"]).T  # [E, N]
    f3pad = np.zeros((E, 48), np.float32)
    for s in range(2):
        # slice s covers psum rows 32k..32k+24; i with s=i%2 lands at +12*s
        f3pad[:, 24 * s + 12 * s : 24 * s + 12 * s + N] = f3WT

    misc = np.zeros((128, MISC_W), np.float32)
    o_ = 0
    misc[:, o_ : o_ + 2] = f32(inputs["sb"]).reshape(EC, 128).T; o_ += 2
    misc[:, o_ : o_ + 2] = f32(inputs["tb"]).reshape(EC, 128).T; o_ += 2
    misc[:, o_ : o_ + 2] = f32(inputs["f2b"]).reshape(EC, 128).T; o_ += 2
    for k in range(4):
        for s in range(2):
            misc[32 * k + 12 * s : 32 * k + 12 * s + N, o_] = f32(inputs["f3b"])
    o_ += 1
    for i in range(128):
        if i % 8 < 6:
            misc[i, o_ + i // 8] = 1.0
    o_ += OCTS
    for i in range(128):
        misc[i, o_ + i % 8] = 1.0
    o_ += 8
    misc[:, o_:] = _chunk_major(f3WT, EC)

    shared = {
        "sWTm": _chunk_major(f32(inputs["sW"]).T, HC).astype(np.float16),
        "tWTm": _chunk_major(f32(inputs["tW"]).T, HC).astype(np.float16),
        "WuTm": _chunk_major(f2W[:, :E].T, EC).astype(np.float16),
        "WvTm": _chunk_major(f2W[:, E:].T, EC).astype(np.float16),
        "blWTm": _chunk_major(f32(inputs["blW"]).T, EC).astype(np.float16),
        "f3padm": _chunk_major(f3pad, EC).astype(np.float16),
        "misc": misc,
    }

    in_maps = []
    for c in range(NCORES):
        b, r = divmod(c, 4)
        m = dict(shared)
        m["xTm"] = _chunk_major(np.ascontiguousarray(x[b].T), HC).astype(np.float16)
        m["xTim"] = _chunk_major(
            np.ascontiguousarray(x[b, IB * r : IB * (r + 1), :].T), HC
        ).astype(np.float16)
        in_maps.append(m)
    return in_maps


def _gather(results):
    full = np.empty((B, L, N, L), np.float32)
    for c in range(NCORES):
        b, r = divmod(c, 4)
        # out is [OCTS, 128, L]; row 32k+12s+n of octet o -> i = 8o+2k+s
        raw = results[c]["out"].astype(np.float32)
        v = raw.reshape(OCTS, 4, 32, L)[:, :, :24, :]
        full[b, IB * r : IB * (r + 1)] = v.reshape(IB, N, L)
    return full


def kernel(x, sW, sb, tW, tb, f2W, f2b, f3W, f3b, blW):
    from concourse.bass_utils import run_bass_kernel_spmd

    in_maps = _make_in_maps(dict(
        x=x, sW=sW, sb=sb, tW=tW, tb=tb, f2W=f2W, f2b=f2b,
        f3W=f3W, f3b=f3b, blW=blW,
    ))
    nc = _get_nc()
    res = run_bass_kernel_spmd(nc, in_maps, core_ids=list(range(NCORES)))
    return _gather(res.results)


# revision 35
# speedup vs baseline: 1.0940x; 1.0815x over previous
"""BiAffine layer kernel for 8 Trainium2 NeuronCores.

Reference computation (per batch b):
  s = relu(x @ sW.T + sb)                  [L, E]
  t = relu(x @ tW.T + tb)                  [L, E]
  key = (s @ blW.T).reshape(L, E, N)
  out1[i, n, l] = sum_e key[i, e, n] * t[l, e]
  su = s @ Wu.T + f2b ; tv = t @ Wv.T      (Wu, Wv = f2W[:, :E], f2W[:, E:])
  h[i, j, :] = relu(su[i] + tv[j])
  out2[i, n, j] = sum_e h[i, j, e] * f3W[n, e] + f3b[n]
  out = out1 + out2                        [L, N, L]

Sharding: 8 cores = 2 batches x 4 blocks of 128 source positions (i).

Octet layout: one PSUM bank [128, 512] holds EIGHT i's: 4 col-groups at
32-aligned offsets, 2 i's packed per group (rows 32k + 12s + n, 8 pad
rows per group).  out1: M=32 matmuls from a zero-padded fp16 key tensor
(also initializes the bank); out2: M=24 matmuls with zero-block-padded
f3W stationaries, 4-way PE column-group concurrency.

h production per octet position p = i%8:
  p=0..5 -> DVE: ONE batched fp16 tensor_tensor per octet computing
    h' = max(tv, -su') for all six i's ([128, 6x512x2] interleaved
    layout, stride-0 broadcast APs; 2x perf mode, ~540ns/i).  The
    dropped +su' makes the matmul short by sum_e f3W[n,e]su'[e,i]; that
    rank-1 correction C is precomputed on the PE and folded (with f3b)
    into the final copy's per-partition bias.
  p=6,7 -> ACT: true h = relu(tv + su') via activation bias, reading tv
    from a persistent PSUM bank (ScalarE's fast src).
Final: one biased copy [128,512] per octet (ACT on even octets, DVE
tensor_scalar-add on odd), then ONE output DMA per octet into a padded
[OCTS, 128, L] fp16 dram layout (host strips the pad rows), issued on
the sync HWDGE ring.  GpSimd does no DMA (SWDGE issue is 2-14us for
large transfers); all input loads ride the sync/scalar HWDGE rings,
s-path tensors first so prep matmuls start during the load tail.
"""

import sys

sys.path.insert(0, "/opt/trn_rl_repo")

import numpy as np

B, L, H, E, N = 2, 512, 768, 256, 12
EC = E // 128  # 2 e-chunks
HC = H // 128  # 6 h-chunks
IB = L // 4  # 128 i's per core
NCORES = 8
OCTS = IB // 8  # 16

# misc fp32 column layout: [sb(2) tb(2) f2b(2) f3b128(1) kxn01(16)
#                           mask8(8) f3WT(24)]
MISC_W = 2 + 2 + 2 + 1 + OCTS + 8 + 2 * N

_cache = {}


def build_nc():
    import concourse.bass as bass
    import concourse.tile as tile
    from concourse import bacc, mybir
    from contextlib import ExitStack

    fp32 = mybir.dt.float32
    fp16 = mybir.dt.float16
    AF = mybir.ActivationFunctionType
    ALU = mybir.AluOpType

    nc = bacc.Bacc("TRN2")

    # ---- I/O (multi-chunk tensors prepacked chunk-major on host) ----
    xTm = nc.dram_tensor("xTm", [128, HC * L], fp16, kind="ExternalInput")
    xTim = nc.dram_tensor("xTim", [128, HC * IB], fp16, kind="ExternalInput")
    sWTm = nc.dram_tensor("sWTm", [128, HC * E], fp16, kind="ExternalInput")
    tWTm = nc.dram_tensor("tWTm", [128, HC * E], fp16, kind="ExternalInput")
    WuTm = nc.dram_tensor("WuTm", [128, EC * E], fp16, kind="ExternalInput")
    WvTm = nc.dram_tensor("WvTm", [128, EC * E], fp16, kind="ExternalInput")
    blWTm = nc.dram_tensor("blWTm", [128, EC * E * N], fp16, kind="ExternalInput")
    f3padm = nc.dram_tensor("f3padm", [128, EC * 48], fp16, kind="ExternalInput")
    misc = nc.dram_tensor("misc", [128, MISC_W], fp32, kind="ExternalInput")
    out = nc.dram_tensor("out", [OCTS, 128, L], fp16, kind="ExternalOutput")

    with tile.TileContext(nc) as tc, ExitStack() as ctx:
        consts = ctx.enter_context(tc.tile_pool(name="consts", bufs=1))
        acts = ctx.enter_context(tc.tile_pool(name="acts", bufs=1))
        tvps = ctx.enter_context(tc.tile_pool(name="tvps", bufs=1, space="PSUM"))

        def load(src, shape, name, dt=fp16, eng=None):
            t = consts.tile(shape, dt, name=name)
            (eng or nc.sync).dma_start(out=t[:], in_=src)
            return t

        # HWDGE rings only; s-path first so prep compute starts early.
        # blW (the biggest tensor) is split across both rings mid-order.
        # t-chain first: tv gates the DVE h stream, the kernel's cadence.
        # xT leads the sync ring (before even misc) so its transfer
        # starts the moment the ring opens.
        xT_m = load(xTm[:], [128, HC * L], "xT_m", eng=nc.sync)
        misc_sb = load(misc[:], [128, MISC_W], "misc_sb", dt=fp32, eng=nc.sync)
        tWT_m = load(tWTm[:], [128, HC * E], "tWT_m", eng=nc.scalar)
        xTi_m = load(xTim[:], [128, HC * IB], "xTi_m", eng=nc.scalar)
        sWT_m = load(sWTm[:], [128, HC * E], "sWT_m", eng=nc.scalar)
        WvT_m = load(WvTm[:], [128, EC * E], "WvT_m", eng=nc.sync)
        WuT_m = load(WuTm[:], [128, EC * E], "WuT_m", eng=nc.scalar)
        blWT_m = consts.tile([128, EC * E * N], fp16, name="blWT_m")
        half = EC * E * N // 2
        nc.sync.dma_start(out=blWT_m[:, :half], in_=blWTm[:, :half])
        nc.scalar.dma_start(out=blWT_m[:, half:], in_=blWTm[:, half:])
        f3pad_m = load(f3padm[:], [128, EC * 48], "f3pad_m", eng=nc.sync)

        xT_sb = [xT_m[:, L * c : L * (c + 1)] for c in range(HC)]
        xTi_sb = [xTi_m[:, IB * c : IB * (c + 1)] for c in range(HC)]
        sWT_sb = [sWT_m[:, E * c : E * (c + 1)] for c in range(HC)]
        tWT_sb = [tWT_m[:, E * c : E * (c + 1)] for c in range(HC)]
        WuT_sb = [WuT_m[:, E * c : E * (c + 1)] for c in range(EC)]
        WvT_sb = [WvT_m[:, E * c : E * (c + 1)] for c in range(EC)]
        blWT_sb = [blWT_m[:, E * N * c : E * N * (c + 1)] for c in range(EC)]
        f3pad_sb = [f3pad_m[:, 48 * c : 48 * (c + 1)] for c in range(EC)]
        o_ = 0
        sb_sb = misc_sb[:, o_ : o_ + 2]; o_ += 2
        tb_sb = misc_sb[:, o_ : o_ + 2]; o_ += 2
        f2b_sb = misc_sb[:, o_ : o_ + 2]; o_ += 2
        f3b_sb = misc_sb[:, o_ : o_ + 1]; o_ += 1
        kxn01_sb = misc_sb[:, o_ : o_ + OCTS]; o_ += OCTS
        mask8_sb = misc_sb[:, o_ : o_ + 8]; o_ += 8
        f3WT_sb = [misc_sb[:, o_ + N * c : o_ + N * (c + 1)] for c in range(EC)]

        # ---- persistent activations ----
        tT_sb, sT_sb, suT_sb, keyE_sb, tvPS = [], [], [], [], []
        for ec in range(EC):
            tT_sb.append(acts.tile([128, L], fp16, name=f"tT{ec}"))
            sT_sb.append(acts.tile([128, IB], fp16, name=f"sT{ec}"))
            suT_sb.append(acts.tile([128, IB], fp32, name=f"suT{ec}"))
            # key, n-major contiguous: col 128*n + i; out1 reads it via a
            # gathered [12 n, 2 i] stationary AP (no scatter, no pads)
            keyE_sb.append(acts.tile([128, N * IB], fp16, name=f"keyE_{ec}"))
            tvPS.append(tvps.tile([128, L], fp32, name=f"tvPS{ec}"))
        tvT2i = acts.tile([128, 2 * L], fp16, name="tvT2i")  # cols 2*j+ec
        negsu2 = acts.tile([128, 2 * IB], fp16, name="negsu2")  # col 2i+ec
        C8sb = acts.tile([128, OCTS], fp32, name="C8sb")
        CT_sb = acts.tile([128, N], fp32, name="CT_sb")
        kxmC = acts.tile([128, 128], fp32, name="kxmC")
        nc.gpsimd.memset(kxmC[:], 0.0)

        # ---- prep ----
        with tc.tile_pool(name="prep_psum", bufs=3, space="PSUM") as pp:
            # PE warmup: junk matmuls while input DMAs land, so HAM
            # unthrottles (1.2 -> 2.4 GHz) before real prep work arrives
            scratch = acts.tile([128, L], fp16, name="scratch")
            nc.gpsimd.memset(scratch[:], 0.0)
            ps_w = pp.tile([128, L], fp32, name="ps_w", tag="ps")
            for w in range(12):
                nc.tensor.matmul(
                    ps_w[:], lhsT=scratch[:, :128], rhs=scratch[:],
                    start=(w == 0), stop=(w == 11))

            # t-chain (tT -> tv gates the main-loop DVE stream)
            tv_i = tvT2i.rearrange("p (f c) -> p f c", c=2)
            for ec in range(EC):
                ps_t = pp.tile([128, L], fp32, name="ps_t", tag="ps")
                for hc in range(HC):
                    nc.tensor.matmul(
                        ps_t[:],
                        lhsT=tWT_sb[hc][:, 128 * ec : 128 * (ec + 1)],
                        rhs=xT_sb[hc],
                        start=(hc == 0),
                        stop=(hc == HC - 1),
                    )
                nc.scalar.activation(tT_sb[ec][:], ps_t[:], AF.Relu,
                                     bias=tb_sb[:, ec : ec + 1])
            for ec in range(EC):
                for epc in range(EC):
                    nc.tensor.matmul(
                        tvPS[ec][:],
                        lhsT=WvT_sb[epc][:, 128 * ec : 128 * (ec + 1)],
                        rhs=tT_sb[epc][:],
                        start=(epc == 0),
                        stop=(epc == EC - 1),
                    )
                nc.vector.tensor_copy(out=tv_i[:, :, ec], in_=tvPS[ec][:])

            # s-chain
            for ec in range(EC):
                ps_s = pp.tile([128, L], fp32, name="ps_s", tag="ps")
                for hc in range(HC):
                    nc.tensor.matmul(
                        ps_s[:, :IB],
                        lhsT=sWT_sb[hc][:, 128 * ec : 128 * (ec + 1)],
                        rhs=xTi_sb[hc],
                        start=(hc == 0),
                        stop=(hc == HC - 1),
                    )
                nc.scalar.activation(sT_sb[ec][:], ps_s[:, :IB], AF.Relu,
                                     bias=sb_sb[:, ec : ec + 1])
            for ec in range(EC):
                # suT = s @ Wu.T + f2b (fp32; per-partition biases/scalars)
                ps_su = pp.tile([128, L], fp32, name="ps_su", tag="ps")
                for epc in range(EC):
                    nc.tensor.matmul(
                        ps_su[:, :IB],
                        lhsT=WuT_sb[epc][:, 128 * ec : 128 * (ec + 1)],
                        rhs=sT_sb[epc][:],
                        start=(epc == 0),
                        stop=(epc == EC - 1),
                    )
                nc.scalar.activation(suT_sb[ec][:], ps_su[:, :IB], AF.Identity,
                                     bias=f2b_sb[:, ec : ec + 1])
                ns2 = negsu2.rearrange("p (i two) -> p i two", two=2)
                nc.vector.tensor_scalar_mul(ns2[:, :, ec], suT_sb[ec][:], -1.0)

            # correction CT[i, n] = sum_e f3WT[e,n] * suT[e,i]  (fp32)
            ps_ct = pp.tile([128, L], fp32, name="ps_ct", tag="ps")
            for ec in range(EC):
                nc.tensor.matmul(
                    ps_ct[:, :N],
                    lhsT=suT_sb[ec][:],
                    rhs=f3WT_sb[ec],
                    start=(ec == 0),
                    stop=(ec == EC - 1),
                )
            nc.vector.tensor_copy(out=CT_sb[:], in_=ps_ct[:, :N])
            # kxmC[:, 32k+12s+n] = CT[:, n] * (i%8 == 2k+s), p<6 only
            for k in range(4):
                for s in range(2):
                    p = 2 * k + s
                    if p >= 6:
                        continue
                    nc.vector.tensor_tensor(
                        out=kxmC[:, 32 * k + 12 * s : 32 * k + 12 * s + N],
                        in0=CT_sb[:],
                        in1=mask8_sb[:, p : p + 1].broadcast_to([128, N]),
                        op=ALU.mult,
                    )
            ps_c8 = pp.tile([128, L], fp32, name="ps_c8", tag="ps")
            nc.tensor.matmul(ps_c8[:, :OCTS], lhsT=kxmC[:], rhs=kxn01_sb,
                             start=True, stop=True)
            nc.vector.tensor_tensor(
                out=C8sb[:], in0=ps_c8[:, :OCTS],
                in1=f3b_sb.broadcast_to([128, OCTS]), op=ALU.add)

        # ---- main loop over octets ----
        # Key production (48 matmuls + 24 strided copies) is interleaved
        # into the first KLAG octets so the DVE h stream never waits on
        # it: those octets accumulate out2 FIRST (start=True, no key
        # dependency) and their out1 joins KLAG octets later
        # (start=False, stop=True).  Octets > KLAG run out1-first
        # (v0-style, start=True incl pad rows) and flush immediately.
        hp = ctx.enter_context(tc.tile_pool(name="hp", bufs=12))
        outp = ctx.enter_context(tc.tile_pool(name="outp", bufs=4))
        mp = ctx.enter_context(tc.tile_pool(name="main_psum", bufs=5, space="PSUM"))
        kp = ctx.enter_context(tc.tile_pool(name="key_psum", bufs=1, space="PSUM"))

        tv_i = tvT2i.rearrange("p (f c) -> p f c", c=2)
        ns2 = negsu2.rearrange("p (i two) -> p i two", two=2)
        blWT3 = [blWT_sb[c].rearrange("p (e n) -> p e n", n=N) for c in range(EC)]
        # key layout: keyF[ec][e, 64*(12s+n) + d]  (i = 2d+s) -> out1's
        # stationary for duo d is a clean 2D AP [24 cols, stride 64]
        keyv = [keyE_sb[c].rearrange("p (c d) -> p c d", d=64)
                for c in range(EC)]
        keyw = [keyE_sb[c].rearrange("p (s n d) -> p s n d", s=2, d=64)
                for c in range(EC)]
        # s-major rhs: key matmul emits cols 64s+d so the copy is
        # contiguous-inner on both sides
        sT_sm = [sT_sb[c].rearrange("p (d s) -> p s d", s=2) for c in range(EC)]
        # 6 key groups of (ec, 4 consecutive n): one PSUM bank + 1 copy each
        KEY_GROUPS = [(ec, 4 * q) for ec in range(EC) for q in range(3)]
        KLAG = 4
        pending = []  # (psum_tile, octet) awaiting lagged out1 + flush

        def key_batch(groups):
            for gi, (ec, n0) in enumerate(groups):
                ps_k = kp.tile([128, L], fp32, name="ps_k", tag="psk")
                for q in range(4):
                    for epc in range(EC):
                        nc.tensor.matmul(
                            ps_k[:, IB * q : IB * (q + 1)],
                            lhsT=blWT3[epc][:, 128 * ec : 128 * (ec + 1),
                                            n0 + q],
                            rhs=sT_sm[epc],
                            start=(epc == 0),
                            stop=(epc == EC - 1),
                        )
                src = ps_k[:].rearrange("p (q s d) -> p s q d", q=4, s=2)
                dst = keyw[ec][:, :, n0 : n0 + 4, :]
                if gi % 2 == 0:
                    nc.vector.tensor_copy(out=dst, in_=src)
                else:
                    nc.scalar.copy(dst, src)

        def emit_out1(ps_o, o, lagged):
            for ec in range(EC):
                for k in range(4):
                    d = 4 * o + k
                    nc.tensor.matmul(
                        ps_o[32 * k : 32 * k + 24, :],
                        lhsT=keyv[ec][:, :, d : d + 1],
                        rhs=tT_sb[ec][:],
                        start=(not lagged and ec == 0),
                        stop=(lagged and ec == EC - 1),
                        tile_position=(0, 32 * k),
                        skip_group_check=True,
                    )

        def flush(ps_o, o):
            ob = outp.tile([128, L], fp16, name="ob")
            if o == OCTS - 1:
                # DVE is idle after its last h6; shaves the tail
                nc.vector.tensor_scalar(
                    out=ob[:], in0=ps_o[:], scalar1=C8sb[:, o : o + 1],
                    scalar2=None, op0=ALU.add)
            else:
                nc.scalar.activation(ob[:], ps_o[:], AF.Identity,
                                     bias=C8sb[:, o : o + 1])
            nc.sync.dma_start(out=out[o], in_=ob[:])

        for o in range(OCTS):
            ps = mp.tile([128, L], fp32, name="ps")
            early = o <= KLAG
            # h: p=0..5 one batched DVE op (h'); p=6,7 ACT (true h, PSUM src)
            i0 = 8 * o
            h6 = hp.tile([128, 6 * 2 * L], fp16, name="h6", tag="h6", bufs=6)
            h6v = h6.rearrange("p (i f c) -> p i f c", f=L, c=2)
            nc.vector.tensor_tensor(
                out=h6v,
                in0=tv_i.unsqueeze(1).broadcast_to([128, 6, L, 2]),
                in1=ns2[:, i0 : i0 + 6, :].unsqueeze(2).broadcast_to(
                    [128, 6, L, 2]),
                op=ALU.max)
            hs = {}
            for p in range(6):
                for ec in range(EC):
                    hs[(p, ec)] = h6v[:, p, :, ec]
            for p in (6, 7):
                i = i0 + p
                for ec in range(EC):
                    ha = hp.tile([128, L], fp16, name="ha", tag="ha", bufs=12)
                    nc.scalar.activation(ha[:], tvPS[ec][:], AF.Relu,
                                         bias=suT_sb[ec][:, i : i + 1])
                    hs[(p, ec)] = ha[:]
            if o < 3:
                key_batch(KEY_GROUPS[2 * o : 2 * o + 2])
            if not early:
                emit_out1(ps, o, lagged=False)
            # out2: M=24 zero-block-padded f3 stationaries; emission order
            # rotates col-groups for PE tile concurrency
            for ec in range(EC):
                for p in (0, 2, 4, 6, 1, 3, 5, 7):
                    k, s = divmod(p, 2)
                    nc.tensor.matmul(
                        ps[32 * k : 32 * k + 24, :],
                        lhsT=f3pad_sb[ec][:, 24 * s : 24 * s + 24],
                        rhs=hs[(p, ec)],
                        start=(early and ec == 0 and s == 0),
                        stop=(not early and ec == EC - 1 and s == 1),
                        tile_position=(0, 32 * k),
                        skip_group_check=True,
                    )
            if early:
                pending.append((ps, o))
                if o == KLAG:
                    ps0, o0 = pending.pop(0)
                    emit_out1(ps0, o0, lagged=True)
                    flush(ps0, o0)
            else:
                flush(ps, o)
                if pending:
                    ps0, o0 = pending.pop(0)
                    emit_out1(ps0, o0, lagged=True)
                    flush(ps0, o0)
        for ps0, o0 in pending:
            emit_out1(ps0, o0, lagged=True)
            flush(ps0, o0)

    nc.compile()
    return nc


def _get_nc():
    if "nc" not in _cache:
        _cache["nc"] = build_nc()
    return _cache["nc"]


def _chunk_major(a, nchunks):
    # [128*nchunks, W] -> [128, nchunks*W] with chunk-major free layout
    W = a.shape[1]
    return np.ascontiguousarray(
        a.reshape(nchunks, 128, W).transpose(1, 0, 2).reshape(128, nchunks * W))


def _make_in_maps(inputs):
    x = np.asarray(inputs["x"], np.float32)
    f32 = lambda a: np.asarray(a, np.float32)

    f2W = f32(inputs["f2W"])
    f3WT = f32(inputs["f3W"]).T  # [E, N]
    f3pad = np.zeros((E, 48), np.float32)
    for s in range(2):
        # slice s covers psum rows 32k..32k+24; i with s=i%2 lands at +12*s
        f3pad[:, 24 * s + 12 * s : 24 * s + 12 * s + N] = f3WT

    misc = np.zeros((128, MISC_W), np.float32)
    o_ = 0
    misc[:, o_ : o_ + 2] = f32(inputs["sb"]).reshape(EC, 128).T; o_ += 2
    misc[:, o_ : o_ + 2] = f32(inputs["tb"]).reshape(EC, 128).T; o_ += 2
    misc[:, o_ : o_ + 2] = f32(inputs["f2b"]).reshape(EC, 128).T; o_ += 2
    for k in range(4):
        for s in range(2):
            misc[32 * k + 12 * s : 32 * k + 12 * s + N, o_] = f32(inputs["f3b"])
    o_ += 1
    for i in range(128):
        if i % 8 < 6:
            misc[i, o_ + i // 8] = 1.0
    o_ += OCTS
    for i in range(128):
        misc[i, o_ + i % 8] = 1.0
    o_ += 8
    misc[:, o_:] = _chunk_major(f3WT, EC)

    shared = {
        "sWTm": _chunk_major(f32(inputs["sW"]).T, HC).astype(np.float16),
        "tWTm": _chunk_major(f32(inputs["tW"]).T, HC).astype(np.float16),
        "WuTm": _chunk_major(f2W[:, :E].T, EC).astype(np.float16),
        "WvTm": _chunk_major(f2W[:, E:].T, EC).astype(np.float16),
        "blWTm": _chunk_major(f32(inputs["blW"]).T, EC).astype(np.float16),
        "f3padm": _chunk_major(f3pad, EC).astype(np.float16),
        "misc": misc,
    }

    in_maps = []
    for c in range(NCORES):
        b, r = divmod(c, 4)
        m = dict(shared)
        m["xTm"] = _chunk_major(np.ascontiguousarray(x[b].T), HC).astype(np.float16)
        m["xTim"] = _chunk_major(
            np.ascontiguousarray(x[b, IB * r : IB * (r + 1), :].T), HC
        ).astype(np.float16)
        in_maps.append(m)
    return in_maps


def _gather(results):
    full = np.empty((B, L, N, L), np.float32)
    for c in range(NCORES):
        b, r = divmod(c, 4)
        # out is [OCTS, 128, L]; row 32k+12s+n of octet o -> i = 8o+2k+s
        raw = results[c]["out"].astype(np.float32)
        v = raw.reshape(OCTS, 4, 32, L)[:, :, :24, :]
        full[b, IB * r : IB * (r + 1)] = v.reshape(IB, N, L)
    return full


def kernel(x, sW, sb, tW, tb, f2W, f2b, f3W, f3b, blW):
    from concourse.bass_utils import run_bass_kernel_spmd

    in_maps = _make_in_maps(dict(
        x=x, sW=sW, sb=sb, tW=tW, tb=tb, f2W=f2W, f2b=f2b,
        f3W=f3W, f3b=f3b, blW=blW,
    ))
    nc = _get_nc()
    res = run_bass_kernel_spmd(nc, in_maps, core_ids=list(range(NCORES)))
    return _gather(res.results)
